# revision 1
# baseline (speedup 1.0000x reference)
"""Bidirectional RNN (tanh) Trainium2 kernel.

Problem: x[32, 2000, 80], h0[32, 512],
  per direction: xp = x @ W_ih.T + b_ih + b_hh  (precomputed bias fold)
  h_t = tanh(xp_t + h_{t-1} @ W_hh.T), scan over t (fwd / reversed)
  out = concat(fwd_states, bwd_states, axis=2) -> [32, 2000, 1024]

Sharding: 8 cores = 4 batch shards (8 batch each) x 2 directions.
The backward direction runs the same forward-scan program on
time-reversed input (host flips t on the way in and out). Fully SPMD.

Per-core layout (hidden-on-partitions; j = jc*128 + p):
  - h tile: [128p, 4jc, 8b]; matmul writes psum[:, jc, 0:8] (bank jc)
  - rhs for recurrent matmul kc is h[:, kc, :] (same layout, no transpose)
  - x fed pre-transposed as [81, 2000, 8] with row 80 = 1.0 so that the
    combined bias rides in W_ihT_aug row 80 (K=81 matmul).
  - per step: 4 xproj matmuls (start=True, independent of h, hide the
    previous step's tanh) + 16 recurrent matmuls + 1 ACT tanh over all
    4 psum banks -> h tile slice. PSUM pool bufs=2 (banks 0-3 / 4-7).
"""

import os
import numpy as np

S = 2000
B = 8  # batch per core
D = 80
H = 512
NCORES = 8
TC = 200  # time steps per hs buffer chunk (DMA-out granularity)
NCHUNK = S // TC

# weights/x/h stream dtype: float32 (exact) or float16 (2x faster PE
# weight load via FWL; psum/tanh stay fp32)
STREAM_NP = np.float16 if os.environ.get("RNN_DT", "fp16") == "fp16" else np.float32

_CACHE = {}


def _build(s=S, tc_steps=TC, stream_np=None):
    import concourse.bass as bass
    import concourse.tile as tile
    from concourse import bacc, mybir

    if stream_np is None:
        stream_np = STREAM_NP
    dt = mybir.dt.from_np(np.dtype(stream_np))
    f32 = mybir.dt.float32
    nchunk = s // tc_steps

    nc = bacc.Bacc("TRN2", target_bir_lowering=False, debug=False)
    xT_d = nc.dram_tensor("xT", [D + 1, s, B], dt, kind="ExternalInput")
    wih_d = nc.dram_tensor("wih", [D + 1, H], dt, kind="ExternalInput")
    whh_d = nc.dram_tensor("whh", [128, 4, H], dt, kind="ExternalInput")
    h0_d = nc.dram_tensor("h0", [128, 4, B], dt, kind="ExternalInput")
    out_d = nc.dram_tensor("out", [128, s, 4, B], dt, kind="ExternalOutput")

    with tile.TileContext(nc) as tc:
        with (
            tc.tile_pool(name="consts", bufs=1) as consts,
            tc.tile_pool(name="hs", bufs=2) as hs_pool,
            tc.tile_pool(name="psum", bufs=2, space="PSUM") as psum_pool,
        ):
            xT_sb = consts.tile([D + 1, s, B], dt)
            wih_sb = consts.tile([D + 1, H], dt)
            whh_sb = consts.tile([128, 4, H], dt)
            h0_sb = consts.tile([128, 4, B], dt)
            nc.sync.dma_start(whh_sb[:], whh_d[:, :, :])
            nc.sync.dma_start(wih_sb[:], wih_d[:, :])
            nc.sync.dma_start(h0_sb[:], h0_d[:, :, :])
            nc.sync.dma_start(xT_sb[:], xT_d[:, :, :])

            prev = h0_sb  # AP provider for h_{t-1}: [:, kc, :] slices
            prev_tl = None  # when prev is an hs tile, the local index
            for c in range(nchunk):
                hs = hs_pool.tile([128, tc_steps, 4, B], dt)
                for tl in range(tc_steps):
                    t = c * tc_steps + tl
                    ps = psum_pool.tile([128, 4, H], f32)
                    # input projection + bias (row 80): no dep on h
                    for jc in range(4):
                        nc.tensor.matmul(
                            ps[:, jc, 0:B],
                            wih_sb[:, jc * 128:(jc + 1) * 128],
                            xT_sb[:, t],
                            start=True,
                            stop=False,
                        )
                    # recurrent part
                    for kc in range(4):
                        if prev_tl is None:
                            rhs = prev[:, kc, :]
                        else:
                            rhs = prev[:, prev_tl, kc, :]
                        for jc in range(4):
                            nc.tensor.matmul(
                                ps[:, jc, 0:B],
                                whh_sb[:, kc, jc * 128:(jc + 1) * 128],
                                rhs,
                                start=False,
                                stop=(kc == 3),
                            )
                    nc.scalar.activation(
                        hs[:, tl],
                        ps[:, :, 0:B],
                        mybir.ActivationFunctionType.Tanh,
                    )
                    prev, prev_tl = hs, tl
                nc.sync.dma_start(out_d[:, c * tc_steps:(c + 1) * tc_steps], hs[:])

    nc.compile()
    return nc


def _get_program(s=S, tc_steps=TC):
    key = (s, tc_steps, np.dtype(STREAM_NP).name)
    if key not in _CACHE:
        _CACHE[key] = _build(s, tc_steps)
    return _CACHE[key]


def _prep_core_inputs(x, h0, W_ih, b_ih, W_hh, b_hh, q, rev, stream_np):
    """Build the in_map for one core: batch quarter q, direction rev."""
    bs = slice(q * B, (q + 1) * B)
    xs = np.asarray(x[bs], np.float32)  # [B, S, D]
    if rev:
        xs = xs[:, ::-1, :]
    xa = np.concatenate([xs, np.ones((B, xs.shape[1], 1), np.float32)], axis=2)
    xT = np.ascontiguousarray(xa.transpose(2, 1, 0)).astype(stream_np)  # [81,S,B]
    wih = np.concatenate(
        [np.asarray(W_ih, np.float32).T, (np.asarray(b_ih, np.float32) + np.asarray(b_hh, np.float32))[None, :]],
        axis=0,
    ).astype(stream_np)  # [81, H]
    whh = (
        np.asarray(W_hh, np.float32).T.reshape(4, 128, H).transpose(1, 0, 2)
    ).astype(stream_np)  # [128, kc, j] = W_hh[j, kc*128+p]
    h0s = (
        np.asarray(h0[bs], np.float32).T.reshape(4, 128, B).transpose(1, 0, 2)
    ).astype(stream_np)  # [128, kc, b]
    return {"xT": np.ascontiguousarray(xT), "wih": wih, "whh": np.ascontiguousarray(whh), "h0": np.ascontiguousarray(h0s)}


def _unshard_core_output(arr, rev):
    """[128, S, 4, B] device layout -> [B, S, H] float32."""
    out = np.asarray(arr, np.float32).transpose(3, 1, 2, 0).reshape(B, -1, H)
    if rev:
        out = out[:, ::-1, :]
    return out


def kernel(x, h0, W_ih_f, b_ih_f, W_hh_f, b_hh_f, W_ih_b, b_ih_b, W_hh_b, b_hh_b):
    from concourse.bass_utils import run_bass_kernel_spmd

    nc = _get_program()
    in_maps = []
    for c in range(NCORES):
        q, rev = c % 4, c >= 4
        if rev:
            W_ih, b_ih, W_hh, b_hh = W_ih_b, b_ih_b, W_hh_b, b_hh_b
        else:
            W_ih, b_ih, W_hh, b_hh = W_ih_f, b_ih_f, W_hh_f, b_hh_f
        in_maps.append(
            _prep_core_inputs(x, h0, W_ih, b_ih, W_hh, b_hh, q, rev, STREAM_NP)
        )
    res = run_bass_kernel_spmd(nc, in_maps, list(range(NCORES))).results
    fwd = np.concatenate([_unshard_core_output(res[q]["out"], False) for q in range(4)], axis=0)
    bwd = np.concatenate([_unshard_core_output(res[4 + q]["out"], True) for q in range(4)], axis=0)
    return np.concatenate([fwd, bwd], axis=2).astype(np.float32)


# revision 4
# speedup vs baseline: 25.3195x; 25.3195x over previous
"""Bidirectional RNN (tanh) Trainium2 kernel.

Problem: x[32, 2000, 80], h0[32, 512],
  per direction: xp = x @ W_ih.T + b_ih + b_hh  (precomputed bias fold)
  h_t = tanh(xp_t + h_{t-1} @ W_hh.T), scan over t (fwd / reversed)
  out = concat(fwd_states, bwd_states, axis=2) -> [32, 2000, 1024]

Sharding: 8 cores = 4 batch shards (8 batch each) x 2 directions.
The backward direction runs the same forward-scan program on
time-reversed input (host flips t on the way in and out). Fully SPMD.

Per-core layout (hidden-on-partitions; j = jc*128 + p):
  - h tile: [128p, 4jc, 8b]; matmul writes psum[:, jc, 0:8] (bank jc)
  - rhs for recurrent matmul kc is h[:, kc, :] (same layout, no transpose)
  - x fed pre-transposed as [81, 2000, 8] with row 80 = 1.0 so that the
    combined bias rides in W_ihT_aug row 80 (K=81 matmul).
  - per step: 4 xproj matmuls (start=True, independent of h, hide the
    previous step's tanh) + 16 recurrent matmuls + 1 ACT tanh over all
    4 psum banks -> h tile slice. PSUM pool bufs=2 (banks 0-3 / 4-7).
"""

import os
import numpy as np

S = 2000
B = 8  # batch per core
D = 80
H = 512
NCORES = 8
TC = 200  # time steps per hs buffer chunk (DMA-out granularity)
NCHUNK = S // TC

# weights/x/h stream dtype: float32 (exact) or float16 (2x faster PE
# weight load via FWL; psum/tanh stay fp32)
STREAM_NP = np.float16 if os.environ.get("RNN_DT", "fp16") == "fp16" else np.float32

_CACHE = {}


def _build(s=S, tc_steps=TC, stream_np=None, repeat=1):
    import contextlib

    import concourse.bass as bass
    import concourse.tile as tile
    from concourse import bacc, mybir

    if stream_np is None:
        stream_np = STREAM_NP
    dt = mybir.dt.from_np(np.dtype(stream_np))
    f32 = mybir.dt.float32
    nchunk = s // tc_steps

    nc = bacc.Bacc("TRN2", target_bir_lowering=False, debug=False)
    xT_d = nc.dram_tensor("xT", [D + 1, s, B], dt, kind="ExternalInput")
    wih_d = nc.dram_tensor("wih", [D + 1, H], dt, kind="ExternalInput")
    whh_d = nc.dram_tensor("whh", [128, 4, H], dt, kind="ExternalInput")
    h0_d = nc.dram_tensor("h0", [128, 4, B], dt, kind="ExternalInput")
    out_d = nc.dram_tensor("out", [128, s, 4, B], dt, kind="ExternalOutput")

    with tile.TileContext(nc) as tc:
        with (
            tc.tile_pool(name="consts", bufs=1) as consts,
            tc.tile_pool(name="hs", bufs=2) as hs_pool,
            tc.tile_pool(name="psum", bufs=2, space="PSUM") as psum_pool,
        ):
            xT_sb = consts.tile([D + 1, s, B], dt)
            wih_sb = consts.tile([D + 1, H], dt)
            whh_sb = consts.tile([128, 4, H], dt)
            h0_sb = consts.tile([128, 4, B], dt)
            nc.sync.dma_start(whh_sb[:], whh_d[:, :, :])
            nc.sync.dma_start(wih_sb[:], wih_d[:, :])
            nc.sync.dma_start(h0_sb[:], h0_d[:, :, :])
            nc.sync.dma_start(xT_sb[:], xT_d[:, :, :])

            # repeat>1 wraps the whole scan in a HW loop (timing only)
            rep_cm = tc.For_i(0, repeat) if repeat > 1 else contextlib.nullcontext()
            with rep_cm:
                prev = h0_sb  # AP provider for h_{t-1}: [:, kc, :] slices
                prev_tl = None  # when prev is an hs tile, the local index
                for c in range(nchunk):
                    hs = hs_pool.tile([128, tc_steps, 4, B], dt)
                    for tl in range(tc_steps):
                        t = c * tc_steps + tl
                        ps = psum_pool.tile([128, 4, H], f32)
                        # input projection + bias (row 80): no dep on h
                        for jc in range(4):
                            nc.tensor.matmul(
                                ps[:, jc, 0:B],
                                wih_sb[:, jc * 128:(jc + 1) * 128],
                                xT_sb[:, t],
                                start=True,
                                stop=False,
                            )
                        # recurrent part
                        for kc in range(4):
                            if prev_tl is None:
                                rhs = prev[:, kc, :]
                            else:
                                rhs = prev[:, prev_tl, kc, :]
                            for jc in range(4):
                                nc.tensor.matmul(
                                    ps[:, jc, 0:B],
                                    whh_sb[:, kc, jc * 128:(jc + 1) * 128],
                                    rhs,
                                    start=False,
                                    stop=(kc == 3),
                                )
                        nc.scalar.activation(
                            hs[:, tl],
                            ps[:, :, 0:B],
                            mybir.ActivationFunctionType.Tanh,
                        )
                        prev, prev_tl = hs, tl
                    nc.sync.dma_start(
                        out_d[:, c * tc_steps:(c + 1) * tc_steps], hs[:]
                    )

    nc.compile()
    return nc


def _get_program(s=S, tc_steps=TC):
    key = (s, tc_steps, np.dtype(STREAM_NP).name)
    if key not in _CACHE:
        _CACHE[key] = _build(s, tc_steps)
    return _CACHE[key]


def _prep_core_inputs(x, h0, W_ih, b_ih, W_hh, b_hh, q, rev, stream_np):
    """Build the in_map for one core: batch quarter q, direction rev."""
    bs = slice(q * B, (q + 1) * B)
    xs = np.asarray(x[bs], np.float32)  # [B, S, D]
    if rev:
        xs = xs[:, ::-1, :]
    xa = np.concatenate([xs, np.ones((B, xs.shape[1], 1), np.float32)], axis=2)
    xT = np.ascontiguousarray(xa.transpose(2, 1, 0)).astype(stream_np)  # [81,S,B]
    wih = np.concatenate(
        [np.asarray(W_ih, np.float32).T, (np.asarray(b_ih, np.float32) + np.asarray(b_hh, np.float32))[None, :]],
        axis=0,
    ).astype(stream_np)  # [81, H]
    whh = (
        np.asarray(W_hh, np.float32).T.reshape(4, 128, H).transpose(1, 0, 2)
    ).astype(stream_np)  # [128, kc, j] = W_hh[j, kc*128+p]
    h0s = (
        np.asarray(h0[bs], np.float32).T.reshape(4, 128, B).transpose(1, 0, 2)
    ).astype(stream_np)  # [128, kc, b]
    return {"xT": np.ascontiguousarray(xT), "wih": wih, "whh": np.ascontiguousarray(whh), "h0": np.ascontiguousarray(h0s)}


def _unshard_core_output(arr, rev):
    """[128, S, 4, B] device layout -> [B, S, H] float32."""
    out = np.asarray(arr, np.float32).transpose(3, 1, 2, 0).reshape(B, -1, H)
    if rev:
        out = out[:, ::-1, :]
    return out


def kernel(x, h0, W_ih_f, b_ih_f, W_hh_f, b_hh_f, W_ih_b, b_ih_b, W_hh_b, b_hh_b):
    from concourse.bass_utils import run_bass_kernel_spmd

    nc = _get_program()
    in_maps = []
    for c in range(NCORES):
        q, rev = c % 4, c >= 4
        if rev:
            W_ih, b_ih, W_hh, b_hh = W_ih_b, b_ih_b, W_hh_b, b_hh_b
        else:
            W_ih, b_ih, W_hh, b_hh = W_ih_f, b_ih_f, W_hh_f, b_hh_f
        in_maps.append(
            _prep_core_inputs(x, h0, W_ih, b_ih, W_hh, b_hh, q, rev, STREAM_NP)
        )
    res = run_bass_kernel_spmd(nc, in_maps, list(range(NCORES))).results
    fwd = np.concatenate([_unshard_core_output(res[q]["out"], False) for q in range(4)], axis=0)
    bwd = np.concatenate([_unshard_core_output(res[4 + q]["out"], True) for q in range(4)], axis=0)
    return np.concatenate([fwd, bwd], axis=2).astype(np.float32)


# revision 5
# speedup vs baseline: 26.0528x; 1.0290x over previous
"""Bidirectional RNN (tanh) Trainium2 kernel.

Problem: x[32, 2000, 80], h0[32, 512],
  per direction: xp = x @ W_ih.T + b_ih + b_hh  (precomputed bias fold)
  h_t = tanh(xp_t + h_{t-1} @ W_hh.T), scan over t (fwd / reversed)
  out = concat(fwd_states, bwd_states, axis=2) -> [32, 2000, 1024]

Sharding: 8 cores = 4 batch shards (8 batch each) x 2 directions.
The backward direction runs the same forward-scan program on
time-reversed input (host flips t on the way in and out). Fully SPMD.

Per-core layout (hidden-on-partitions; j = jc*128 + p):
  - h tile: [128p, 4jc, 8b]; matmul writes psum[:, jc, 0:8] (bank jc)
  - rhs for recurrent matmul kc is h[:, kc, :] (same layout, no transpose)
  - x fed pre-transposed as [81, 2000, 8] with row 80 = 1.0 so that the
    combined bias rides in W_ihT_aug row 80 (K=81 matmul).
  - per step: 4 xproj matmuls (start=True, independent of h, hide the
    previous step's tanh) + 16 recurrent matmuls + 1 ACT tanh over all
    4 psum banks -> h tile slice. PSUM pool bufs=2 (banks 0-3 / 4-7).
"""

import os
import numpy as np

S = 2000
B = 8  # batch per core
D = 80
H = 512
NCORES = 8
TC = 200  # time steps per hs buffer chunk (DMA-out granularity)
NCHUNK = S // TC

# weights/x/h stream dtype: float32 (exact) or float16 (2x faster PE
# weight load via FWL; psum/tanh stay fp32)
STREAM_NP = np.float16 if os.environ.get("RNN_DT", "fp16") == "fp16" else np.float32

_CACHE = {}


def _build(s=S, tc_steps=TC, stream_np=None, repeat=1):
    import contextlib

    import concourse.tile as tile
    from concourse import bacc, mybir

    if stream_np is None:
        stream_np = STREAM_NP
    dt = mybir.dt.from_np(np.dtype(stream_np))
    f32 = mybir.dt.float32
    nchunk = s // tc_steps

    nc = bacc.Bacc("TRN2", target_bir_lowering=False, debug=False)
    xT_d = nc.dram_tensor("xT", [D + 1, s, B], dt, kind="ExternalInput")
    wih_d = nc.dram_tensor("wih", [D + 1, H], dt, kind="ExternalInput")
    whh_d = nc.dram_tensor("whh", [128, 4, H], dt, kind="ExternalInput")
    h0_d = nc.dram_tensor("h0", [128, 4, B], dt, kind="ExternalInput")
    out_d = nc.dram_tensor("out", [128, s, 4, B], dt, kind="ExternalOutput")

    with tile.TileContext(nc) as tc:
        with (
            tc.tile_pool(name="consts", bufs=1) as consts,
            tc.tile_pool(name="hs", bufs=2) as hs_pool,
            tc.tile_pool(name="psum", bufs=2, space="PSUM") as psum_pool,
        ):
            xT_sb = consts.tile([D + 1, s, B], dt)
            wih_sb = consts.tile([D + 1, H], dt)
            whh_sb = consts.tile([128, 4, H], dt)
            h0_sb = consts.tile([128, 4, B], dt)
            nc.sync.dma_start(whh_sb[:], whh_d[:, :, :])
            nc.sync.dma_start(wih_sb[:], wih_d[:, :])
            nc.sync.dma_start(h0_sb[:], h0_d[:, :, :])
            nc.sync.dma_start(xT_sb[:], xT_d[:, :, :])

            # repeat>1 wraps the whole scan in a HW loop (timing only)
            rep_cm = tc.For_i(0, repeat) if repeat > 1 else contextlib.nullcontext()
            with rep_cm:
                prev = h0_sb  # AP provider for h_{t-1}: [:, kc, :] slices
                prev_tl = None  # when prev is an hs tile, the local index
                for c in range(nchunk):
                    hs = hs_pool.tile([128, tc_steps, 4, B], dt)
                    for tl in range(tc_steps):
                        t = c * tc_steps + tl
                        ps = psum_pool.tile([128, 4, H], f32)
                        # input projection + bias (row 80): no dep on h
                        for jc in range(4):
                            nc.tensor.matmul(
                                ps[:, jc, 0:B],
                                wih_sb[:, jc * 128:(jc + 1) * 128],
                                xT_sb[:, t],
                                start=True,
                                stop=False,
                            )
                        # recurrent part
                        for kc in range(4):
                            if prev_tl is None:
                                rhs = prev[:, kc, :]
                            else:
                                rhs = prev[:, prev_tl, kc, :]
                            for jc in range(4):
                                nc.tensor.matmul(
                                    ps[:, jc, 0:B],
                                    whh_sb[:, kc, jc * 128:(jc + 1) * 128],
                                    rhs,
                                    start=False,
                                    stop=(kc == 3),
                                )
                        nc.scalar.activation(
                            hs[:, tl],
                            ps[:, :, 0:B],
                            mybir.ActivationFunctionType.Tanh,
                        )
                        prev, prev_tl = hs, tl
                    nc.sync.dma_start(
                        out_d[:, c * tc_steps:(c + 1) * tc_steps], hs[:]
                    )

    nc.compile()
    return nc


def _get_program(s=S, tc_steps=TC):
    key = (s, tc_steps, np.dtype(STREAM_NP).name)
    if key not in _CACHE:
        _CACHE[key] = _build(s, tc_steps)
    return _CACHE[key]


def _prep_core_inputs(x, h0, W_ih, b_ih, W_hh, b_hh, q, rev, stream_np):
    """Build the in_map for one core: batch quarter q, direction rev."""
    bs = slice(q * B, (q + 1) * B)
    xs = np.asarray(x[bs], np.float32)  # [B, S, D]
    if rev:
        xs = xs[:, ::-1, :]
    xa = np.concatenate([xs, np.ones((B, xs.shape[1], 1), np.float32)], axis=2)
    xT = np.ascontiguousarray(xa.transpose(2, 1, 0)).astype(stream_np)  # [81,S,B]
    wih = np.concatenate(
        [np.asarray(W_ih, np.float32).T, (np.asarray(b_ih, np.float32) + np.asarray(b_hh, np.float32))[None, :]],
        axis=0,
    ).astype(stream_np)  # [81, H]
    whh = (
        np.asarray(W_hh, np.float32).T.reshape(4, 128, H).transpose(1, 0, 2)
    ).astype(stream_np)  # [128, kc, j] = W_hh[j, kc*128+p]
    h0s = (
        np.asarray(h0[bs], np.float32).T.reshape(4, 128, B).transpose(1, 0, 2)
    ).astype(stream_np)  # [128, kc, b]
    return {"xT": np.ascontiguousarray(xT), "wih": wih, "whh": np.ascontiguousarray(whh), "h0": np.ascontiguousarray(h0s)}


def _unshard_core_output(arr, rev):
    """[128, S, 4, B] device layout -> [B, S, H] float32."""
    out = np.asarray(arr, np.float32).transpose(3, 1, 2, 0).reshape(B, -1, H)
    if rev:
        out = out[:, ::-1, :]
    return out


def kernel(x, h0, W_ih_f, b_ih_f, W_hh_f, b_hh_f, W_ih_b, b_ih_b, W_hh_b, b_hh_b):
    from concourse.bass_utils import run_bass_kernel_spmd

    nc = _get_program()
    in_maps = []
    for c in range(NCORES):
        q, rev = c % 4, c >= 4
        if rev:
            W_ih, b_ih, W_hh, b_hh = W_ih_b, b_ih_b, W_hh_b, b_hh_b
        else:
            W_ih, b_ih, W_hh, b_hh = W_ih_f, b_ih_f, W_hh_f, b_hh_f
        in_maps.append(
            _prep_core_inputs(x, h0, W_ih, b_ih, W_hh, b_hh, q, rev, STREAM_NP)
        )
    res = run_bass_kernel_spmd(nc, in_maps, list(range(NCORES))).results
    fwd = np.concatenate([_unshard_core_output(res[q]["out"], False) for q in range(4)], axis=0)
    bwd = np.concatenate([_unshard_core_output(res[4 + q]["out"], True) for q in range(4)], axis=0)
    return np.concatenate([fwd, bwd], axis=2).astype(np.float32)
